# revision 16
# baseline (speedup 1.0000x reference)
"""GQA attention kernel for 8 Trainium2 NeuronCores.

Sharding: batch x head-group. Core c handles batch b = c // 4 and head
group g = c % 4 (8 q heads 8g..8g+7, kv heads 2g, 2g+1). Each core
computes a partial output  attn_out_g[b] @ w_out[rows of g]  and the
host sums the 4 partials per batch.

v4: phase-serial (projection -> attention -> out-projection) like the
original baseline -- interleaving phases on the PE measured slower --
with these fixes:
  * startup: x DMA-transposes ride the SP HWDGE ring while resident
    weights load on the ACT HWDGE ring, ordered by first use
  * attention: per-head score tiles (1 PSUM bank, 3 bufs) decouple the
    scores->exp->PV chain; exp runs per head; PV trails scores by one
    k-tile; normalization trails one row and reads the PV PSUM directly
    (cross-partition-base DVE multiply, no mvB matmul, no staging
    copies)
  * out-projection: PSUM->SBUF copies on the (idle) ACT engine, output
    DMA on the SWDGE ring
"""

import numpy as np
import ml_dtypes

B, T, D = 2, 2048, 2048
H, KVH, HD = 32, 8, 64
KVD = KVH * HD  # 512
NCORES = 8
SCALE = 1.0 / np.sqrt(HD)

_CACHE = {}


def _build():
    import concourse.bass as bass
    import concourse.mybir as mybir
    import concourse.tile as tile
    from concourse import bacc

    f32 = mybir.dt.float32
    bf16 = mybir.dt.bfloat16
    AF = mybir.ActivationFunctionType
    OP = mybir.AluOpType

    nc = bacc.Bacc("TRN2", target_bir_lowering=False, debug=False)

    xb = nc.dram_tensor("xb", [D, T], bf16, kind="ExternalInput")  # x^T, host-transposed
    wqk = nc.dram_tensor("wqk", [D, 640], bf16, kind="ExternalInput")
    wv = nc.dram_tensor("wv", [D, 128], bf16, kind="ExternalInput")
    wo = nc.dram_tensor("wo", [512, D], bf16, kind="ExternalInput")
    sinT = nc.dram_tensor("sinT", [128, T], bf16, kind="ExternalInput")
    cosT = nc.dram_tensor("cosT", [128, T], bf16, kind="ExternalInput")
    perm = nc.dram_tensor("perm", [128, 128], bf16, kind="ExternalInput")
    ident128 = nc.dram_tensor("ident128", [128, 128], bf16, kind="ExternalInput")
    masks = nc.dram_tensor("masks", [4 * 128, 512], bf16, kind="ExternalInput")
    outp = nc.dram_tensor("outp", [T, D], f32, kind="ExternalOutput")

    DT = D // 128   # 16 d-tiles
    NSLAB = 4       # token slabs of 512
    SLAB = 512
    NKT = T // 128  # 16 k token tiles

    with tile.TileContext(nc) as tc:
        with (
            tc.tile_pool(name="const", bufs=1) as cpool,
            tc.tile_pool(name="resid", bufs=1) as rpool,
            tc.tile_pool(name="pa", bufs=2) as pa,
        ):
            # ---- resident constants ----
            wqk_sb = [cpool.tile([128, 640], bf16, tag=f"wqk{i}", name=f"wqk{i}") for i in range(DT)]
            wv_sb = [cpool.tile([128, 128], bf16, tag=f"wv{i}", name=f"wv{i}") for i in range(DT)]
            wo_sb = [cpool.tile([128, D], bf16, tag=f"wo{i}", name=f"wo{i}") for i in range(4)]
            sin_sb = cpool.tile([128, T], bf16, tag="sin")
            cos_sb = cpool.tile([128, T], bf16, tag="cos")
            perm_sb = cpool.tile([128, 128], bf16, tag="perm")
            id128_sb = cpool.tile([128, 128], bf16, tag="id128")
            mask_sb = [cpool.tile([128, 512], bf16, tag=f"mask{r}", name=f"mask{r}") for r in range(4)]
            ones_sb = cpool.tile([1, 64], bf16, tag="ones")

            # single (SP) HWDGE ring, ordered by first use: slab-0
            # transposes interleaved with the wqk tiles they pair with in
            # the first projection chain; wo loads are emitted after phase
            # A so they queue behind the later slab transposes.
            wqk3 = wqk.rearrange("(o p) e -> p o e", p=128)
            wv3 = wv.rearrange("(o p) e -> p o e", p=128)
            wo3 = wo.rearrange("(o p) e -> p o e", p=128)
            xT0 = [pa.tile([128, SLAB], bf16, tag=f"xT{d}", name=f"xT{d}_0")
                   for d in range(DT)]
            for d in range(DT):
                nc.sync.dma_start(wqk_sb[d][:], wqk3[:, d])
                nc.sync.dma_start(
                    xT0[d][:], xb[d * 128:(d + 1) * 128, 0:SLAB])
            nc.gpsimd.dma_start(sin_sb[:], sinT[:])
            nc.gpsimd.dma_start(cos_sb[:], cosT[:])
            nc.gpsimd.dma_start(perm_sb[:], perm[:])
            for i in range(DT):
                nc.gpsimd.dma_start(wv_sb[i][:], wv3[:, i])
            nc.gpsimd.dma_start(id128_sb[:], ident128[:])
            m4 = masks.rearrange("(r p) q -> r p q", p=128)
            for r in range(4):
                nc.gpsimd.dma_start(mask_sb[r][:], m4[r])
            for i in range(4):
                nc.gpsimd.dma_start(wo_sb[i][:], wo3[:, i])
            nc.gpsimd.memset(ones_sb[:], 1.0)

            # ---- persistent activations ----
            # qT tiles j=0..3: partitions 0:64 head j, 64:128 head j+4
            qkT = [rpool.tile([128, T], bf16, tag=f"qkT{e}", name=f"qkT{e}") for e in range(5)]
            vnat = [rpool.tile([128, 130], bf16, tag=f"vn{k}", name=f"vn{k}") for k in range(NKT)]
            attnT = [rpool.tile([128, T], bf16, tag=f"attnT{j}", name=f"attnT{j}") for j in range(4)]
            for k in range(NKT):
                nc.gpsimd.memset(vnat[k][:], 1.0)

            # ================= Phase A: projections =================
            with (
                tc.tile_pool(name="parope", bufs=3) as pr,
                tc.tile_pool(name="ps_qkv", bufs=2, space="PSUM") as ps_qkv,
                tc.tile_pool(name="ps_rot", bufs=2, space="PSUM") as ps_rot,
                tc.tile_pool(name="ps_v", bufs=2, space="PSUM") as ps_v,
            ):
                for s in range(NSLAB):
                    if s == 0:
                        xT = xT0
                    else:
                        xT = [pa.tile([128, SLAB], bf16, tag=f"xT{d}", name=f"xT{d}_{s}") for d in range(DT)]
                        for d in range(DT):
                            nc.sync.dma_start(
                                xT[d][:], xb[d * 128:(d + 1) * 128, s * SLAB:(s + 1) * SLAB]
                            )
                    # q/k/v projection (transposed out) + rope
                    for e in range(6):
                        acc = ps_qkv.tile([128, SLAB], f32, tag="qkv")
                        wsrc = wqk_sb if e < 5 else wv_sb
                        ecol = e * 128 if e < 5 else 0
                        for d in range(DT):
                            nc.tensor.matmul(
                                acc[:], wsrc[d][:, ecol:ecol + 128], xT[d][:],
                                start=(d == 0), stop=(d == DT - 1),
                            )
                        raw = pr.tile([128, SLAB], bf16, tag="raw")
                        nc.vector.tensor_copy(raw[:], acc[:])
                        if e == 5:
                            # vT -> PE transpose -> v natural (+ ones cols)
                            for i in range(4):
                                kt = 4 * s + i
                                vtp = ps_v.tile([128, 128], bf16, tag="v")
                                nc.tensor.transpose(
                                    vtp[:], raw[:, i * 128:(i + 1) * 128], id128_sb[:])
                                nc.vector.tensor_copy(vnat[kt][:, 0:64], vtp[:, 0:64])
                                nc.vector.tensor_copy(vnat[kt][:, 65:129], vtp[:, 64:128])
                            continue
                        rot = ps_rot.tile([128, SLAB], f32, tag="rot")
                        nc.tensor.matmul(rot[:], perm_sb[:], raw[:], start=True, stop=True)
                        m2 = pr.tile([128, SLAB], bf16, tag="m2")
                        nc.vector.tensor_tensor(
                            m2[:], raw[:], cos_sb[:, s * SLAB:(s + 1) * SLAB], OP.mult)
                        m1 = pr.tile([128, SLAB], bf16, tag="m1")
                        nc.vector.tensor_tensor(
                            m1[:], rot[:], sin_sb[:, s * SLAB:(s + 1) * SLAB], OP.mult)
                        nc.vector.tensor_tensor(
                            qkT[e][:, s * SLAB:(s + 1) * SLAB], m1[:], m2[:], OP.add)

            # ================= Phase B: attention =================
            with (
                tc.tile_pool(name="pb", bufs=4) as pb,
                tc.tile_pool(name="pbn", bufs=3) as pbn,
                tc.tile_pool(name="pc", bufs=3) as pc,
                tc.tile_pool(name="ps_sc", bufs=4, space="PSUM") as ps_sc,
                tc.tile_pool(name="ps_pv", bufs=4, space="PSUM") as ps_pv,
            ):
                def emit_norm(pend):
                    den, pv, pvB, jj, qq = pend
                    qsl2 = slice(qq * 512, (qq + 1) * 512)
                    bcp = ps_sc.tile([128, 512], f32, tag="sc")
                    nc.tensor.matmul(
                        bcp[0:64, :], ones_sb[0:1, :], den[0:1, 0:512],
                        start=True, stop=True)
                    nc.tensor.matmul(
                        bcp[64:128, :], ones_sb[0:1, :], den[0:1, 512:1024],
                        start=True, stop=True)
                    rec = pbn.tile([128, 512], f32, tag="rec")
                    nc.vector.reciprocal_approx_fast(rec[:], bcp[:])
                    nc.vector.tensor_tensor(
                        attnT[jj][0:64, qsl2], pv[0:64, :], rec[0:64, :], OP.mult)
                    nc.vector.tensor_tensor(
                        attnT[jj][64:128, qsl2], pvB[0:64, :], rec[64:128, :], OP.mult)

                def phaseC(s):
                    for i in range(4 * s, 4 * s + 4):
                        for ns in range(4):
                            po = ps_sc.tile([128, 512], f32, tag="sc")
                            for j in range(4):
                                nc.tensor.matmul(
                                    po[:],
                                    attnT[j][:, i * 128:(i + 1) * 128],
                                    wo_sb[j][:, ns * 512:(ns + 1) * 512],
                                    start=(j == 0), stop=(j == 3),
                                )
                            ot = pc.tile([128, 512], f32, tag="ot")
                            nc.vector.tensor_copy(ot[:], po[:])
                            nc.gpsimd.dma_start(
                                outp[i * 128:(i + 1) * 128, ns * 512:(ns + 1) * 512],
                                ot[:])

                pending = None
                for qs in range(4):
                    for j in range(4):
                        if qs >= 1 and j == 1:
                            phaseC(qs - 1)
                        nkt = 4 * qs + 4
                        qsl = slice(qs * 512, (qs + 1) * 512)
                        pv = ps_pv.tile([128, 512], f32, tag="pv")
                        pvB = ps_pv.tile([128, 512], f32, tag="pv")
                        probs = {}
                        # software pipeline: scores/exp one kt ahead of PV
                        for kt in range(nkt + 1):
                            if kt == 2 and pending is not None:
                                emit_norm(pending)
                                pending = None
                            if kt < nkt:
                                # diagonal tiles: query cols < 128r are
                                # causally dead -- skip them in scores/exp
                                # and zero-fill so PV sees zeros.
                                r = kt - 4 * qs
                                off = 128 * r if r > 0 else 0
                                w = 512 - off
                                q0 = qs * 512 + off
                                scA = ps_sc.tile([128, 512], f32, tag="sc")
                                scB = ps_sc.tile([128, 512], f32, tag="sc")
                                nc.tensor.matmul(
                                    scA[:, 0:w], qkT[4][0:64, kt * 128:(kt + 1) * 128],
                                    qkT[j][0:64, q0:(qs + 1) * 512],
                                    start=True, stop=True)
                                nc.tensor.matmul(
                                    scB[:, 0:w], qkT[4][64:128, kt * 128:(kt + 1) * 128],
                                    qkT[j][64:128, q0:(qs + 1) * 512],
                                    start=True, stop=True)
                                pk = pb.tile([128, 1024], bf16, tag="probs")
                                if off:
                                    nc.gpsimd.memset(pk[:, 0:off], 0.0)
                                    nc.gpsimd.memset(pk[:, 512:512 + off], 0.0)
                                nc.scalar.activation(
                                    pk[:, off:512], scA[:, 0:w], AF.Exp,
                                    scale=float(SCALE))
                                nc.scalar.activation(
                                    pk[:, 512 + off:1024], scB[:, 0:w], AF.Exp,
                                    scale=float(SCALE))
                                if r >= 0:
                                    nc.vector.tensor_tensor(
                                        pk[:, off:512], pk[:, off:512],
                                        mask_sb[r][:, off:512], OP.mult)
                                    nc.vector.tensor_tensor(
                                        pk[:, 512 + off:1024], pk[:, 512 + off:1024],
                                        mask_sb[r][:, off:512], OP.mult)
                                probs[kt] = pk
                            if kt >= 1:
                                k0 = kt - 1
                                r0 = k0 - 4 * qs
                                off0 = 128 * r0 if r0 > 0 else 0
                                nc.tensor.matmul(
                                    pv[0:65, off0:512], vnat[k0][:, 0:65],
                                    probs[k0][:, off0:512],
                                    start=(k0 == 0), stop=(k0 == nkt - 1),
                                )
                                nc.tensor.matmul(
                                    pvB[0:65, off0:512], vnat[k0][:, 65:130],
                                    probs[k0][:, 512 + off0:1024],
                                    start=(k0 == 0), stop=(k0 == nkt - 1),
                                )
                        # denominator rows out of PSUM for the pending norm
                        den = pbn.tile([1, 1024], bf16, tag="den")
                        nc.vector.tensor_copy(den[0:1, 0:512], pv[64:65, :])
                        nc.vector.tensor_copy(den[0:1, 512:1024], pvB[64:65, :])
                        pending = (den, pv, pvB, j, qs)
                emit_norm(pending)
                phaseC(3)

    nc.finalize()
    return nc


def _host_inputs(x, sin, cos, w_qkv, w_out):
    bf = ml_dtypes.bfloat16
    sinT_np = np.concatenate([sin.T, sin.T], axis=0).astype(bf)  # [128, T]
    cosT_np = np.concatenate([cos.T, cos.T], axis=0).astype(bf)

    perm_np = np.zeros((128, 128), np.float32)
    for blk in range(2):
        for p in range(64):
            k = blk * 64 + ((p + 32) % 64)
            perm_np[k, blk * 64 + p] = -1.0 if p < 32 else 1.0
    perm_np = perm_np.astype(bf)
    id128_np = np.eye(128, dtype=np.float32).astype(bf)

    mask_np = np.zeros((4, 128, 512), np.float32)
    cix = np.arange(512)[None, :]
    pix = np.arange(128)[:, None]
    for r in range(4):
        mask_np[r] = (cix >= 128 * r + pix).astype(np.float32)
    mask_np = mask_np.reshape(512, 512).astype(bf)

    in_maps = []
    for c in range(NCORES):
        b, g = divmod(c, 4)
        cols = []
        for j in range(4):
            h1, h2 = 8 * g + j, 8 * g + 4 + j
            cols.append(w_qkv[:, 64 * h1:64 * h1 + 64])
            cols.append(w_qkv[:, 64 * h2:64 * h2 + 64])
        cols.append(w_qkv[:, D + 128 * g: D + 128 * g + 128])  # k heads 2g,2g+1
        wqk_np = np.concatenate(cols, axis=1).astype(bf)
        wv_np = w_qkv[:, D + KVD + 128 * g: D + KVD + 128 * g + 128].astype(bf)
        rows = []
        for j in range(4):
            h1, h2 = 8 * g + j, 8 * g + 4 + j
            rows.append(w_out[64 * h1:64 * h1 + 64, :])
            rows.append(w_out[64 * h2:64 * h2 + 64, :])
        wo_np = np.concatenate(rows, axis=0).astype(bf)
        in_maps.append({
            "xb": np.ascontiguousarray(x[b].T).astype(bf),
            "wqk": wqk_np,
            "wv": wv_np,
            "wo": wo_np,
            "sinT": sinT_np,
            "cosT": cosT_np,
            "perm": perm_np,
            "ident128": id128_np,
            "masks": mask_np,
        })
    return in_maps


def kernel(x, sin, cos, w_qkv, w_out, _trace=False):
    from concourse.bass_utils import run_bass_kernel_spmd

    if "nc" not in _CACHE:
        _CACHE["nc"] = _build()
    nc = _CACHE["nc"]

    in_maps = _host_inputs(
        np.asarray(x), np.asarray(sin), np.asarray(cos),
        np.asarray(w_qkv), np.asarray(w_out))
    res = run_bass_kernel_spmd(
        nc, in_maps, core_ids=list(range(NCORES)), trace=_trace)
    out = np.zeros((B, T, D), np.float32)
    for c in range(NCORES):
        b = c // 4
        out[b] += res.results[c]["outp"]
    if _trace:
        kernel.last_result = res
    return out
